# revision 22
# baseline (speedup 1.0000x reference)
"""ContextGuidedTokenShift Trainium2 kernel (v2: fp16 I/O + w-folded matrices).

Full-input contract: kernel(x=(8,16384,576) f32, weight=() f32) -> (8,16384,576) f32.

Math (per batch b, H=W=128, token n = y*128 + xx):
    out[n, c] = w * shifted[n, c] + (1-w) * x[n, c]
    shifted[y*128+xx, c] = x[(y-dy)*128 + (xx-dx), c]  for c in slab (dy, dx),
                           0 where y-dy or xx-dx falls outside [0, 128).

Sharding: pure data-parallel over batch; core i processes x[i].

I/O in fp16 (host casts f32<->fp16; tolerance is 2e-2, fp16 keeps ~1e-3):
halves HBM traffic vs f32 -> per-core DMA floor ~105us at ~360 GB/s.

Layout: SBUF partition p = image row y; free dim = (token, channel) chunks of
16 tokens; all 8 chunk tiles live in SBUF at once (147KB/partition) so every
in-DMA issues immediately and cross-chunk token reads need no buffer rotation.

Channel slabs (c0, cw per offset):
  A (dy==0, dx!=0): c 0:128, 256:320  -> DVE: prefill (1-w)x (tensor_scalar
    @4x), then in-place scalar_tensor_tensor blend from dx-shifted SBUF views.
  B dx==0 (dy!=0):  c 128:256, 320:384 -> TensorE with FOLDED stationary
    M_dy = w*S_dy + (1-w)*I: PSUM = final out; ACT copies PSUM->out (fp16).
  B diag (dy,dx!=0): c 384:576 -> TensorE with wS_dy = w*S_dy: PSUM =
    w*shifted; ACT copies PSUM->ws staging; DVE adds ws into prefilled
    (1-w)x output region (tensor_tensor add @2x).

The folded/scaled matrices are built on device from inline S_dy / I constants
and the runtime weight (w broadcast per-partition).  Out-of-range rows (y
shift) come out as (1-w)x via the matrices' zero rows; out-of-range tokens (x
shift) use zero moving operands (diag) or skipped blend regions (A slabs).

Engine budget per core (target wall ~110us): DMA ~105us (floor), ACT ~48us
(PSUM copies), DVE ~50us (prefill 4x + A blends 1x + diag adds 2x), PE
~55-80us (fp16 matmuls), Pool: out-DMA dispatch only.
"""

import numpy as np

B, H, W, C = 8, 128, 128, 576
IO_NP_DT = np.float16  # device I/O dtype; host casts f32<->fp16 (tol 2e-2)
N = H * W
NCORES = 8
CHUNK = 16            # tokens per tile
NCHUNK = W // CHUNK   # 8 tiles per core
GT = 8                # tokens per PSUM group
NG = CHUNK // GT
FD = CHUNK * C        # 9216 free elements per tile
DIAG_C0, DIAG_CW = 384, 192   # diagonal slabs channel range
WS_FD = CHUNK * DIAG_CW       # ws staging tile free size

# slabs: (dy, dx, c0, cw); cw = 64 // (|dy|+|dx|)
_OFFSETS = [(0, 1), (0, -1), (1, 0), (-1, 0), (0, 2), (0, -2), (2, 0), (-2, 0),
            (1, 1), (-1, -1), (1, -1), (-1, 1), (2, 2), (-2, -2), (2, -2), (-2, 2)]


def _build_slabs():
    slabs, c = [], 0
    for dy, dx in _OFFSETS:
        cw = 64 // (abs(dy) + abs(dx))
        slabs.append((dy, dx, c, cw))
        c += cw
    assert c == C
    return slabs


SLABS = _build_slabs()
A_SLABS = [s for s in SLABS if s[0] == 0]                 # dy == 0
BF_SLABS = [s for s in SLABS if s[0] != 0 and s[1] == 0]  # dy!=0, dx==0: folded
BD_SLABS = sorted([s for s in SLABS if s[0] != 0 and s[1] != 0],
                  key=lambda s: s[0])                     # diagonals, by dy
# PSUM regions per 8-token group, (c, t) layout: idx = (c - base)*GT + t
#   b1: c in [128,256) (folded dy+-1), 1024 f32, 2 banks, double buffered
#   b2: c in [320,576): [0:512) folded dy+-2, [512:2048) diagonals; 4 banks
B1_FD, B2_FD = 1024, 2048


def _psum_region(c0):
    if 128 <= c0 < 256:
        return "b1", (c0 - 128) * GT
    assert 320 <= c0 < 576
    return "b2", (c0 - 320) * GT


def _shift_matrix(dy):
    m = np.zeros((128, 128), np.float32)
    for p in range(128):
        q = p - dy
        if 0 <= q < 128:
            m[q, p] = 1.0
    return m


_CACHE = {}


def _build_bass():
    import concourse.bacc as bacc
    import concourse.mybir as mybir
    from concourse.tile import TileContext

    f32 = mybir.dt.float32
    f16 = mybir.dt.from_np(np.dtype(IO_NP_DT))
    MULT = mybir.AluOpType.mult
    ADD = mybir.AluOpType.add

    nc = bacc.Bacc("TRN2", target_bir_lowering=False, debug=False,
                   num_devices=NCORES)

    x_d = nc.dram_tensor("x", [N, C], f16, kind="ExternalInput")
    w_d = nc.dram_tensor("weight", [128, 1], f32, kind="ExternalInput")
    o_d = nc.dram_tensor("out", [N, C], f16, kind="ExternalOutput")

    # [y, (token, channel)] views: per-partition rows are contiguous in DRAM
    x_row = x_d.ap().rearrange("(y u) c -> y (u c)", y=128)
    o_row = o_d.ap().rearrange("(y u) c -> y (u c)", y=128)

    shift_dram = {dy: nc.inline_tensor(_shift_matrix(dy).astype(IO_NP_DT),
                                       name=f"shm{dy}")
                  for dy in (1, -1, 2, -2)}
    ident_dram = nc.inline_tensor(np.eye(128, dtype=IO_NP_DT), name="ident")

    with TileContext(nc) as tc:
        with (
            tc.tile_pool(name="const", bufs=1) as cpool,
            tc.tile_pool(name="xin", bufs=NCHUNK) as xpool,
            tc.tile_pool(name="oot", bufs=2) as opool,
            tc.tile_pool(name="wst", bufs=4) as wspool,
            tc.tile_pool(name="psb1", bufs=2, space="PSUM") as psb1pool,
            tc.tile_pool(name="psb2", bufs=1, space="PSUM") as psb2pool,
        ):
            w_sb = cpool.tile([128, 1], f32, tag="w", name="w_sb")
            w1_sb = cpool.tile([128, 1], f32, tag="w1", name="w1_sb")
            nc.sync.dma_start(out=w_sb, in_=w_d.ap())
            nc.vector.tensor_scalar(out=w1_sb, in0=w_sb, scalar1=-1.0,
                                    scalar2=1.0, op0=MULT, op1=ADD)

            id_sb = cpool.tile([128, 128], f16, tag="id", name="id_sb")
            nc.sync.dma_start(out=id_sb, in_=ident_dram.ap())
            wi_sb = cpool.tile([128, 128], f16, tag="wi", name="wi_sb")
            nc.scalar.mul(wi_sb, id_sb, w1_sb[:, 0:1])      # (1-w) * I

            wS, M = {}, {}
            for dy in (1, -1, 2, -2):
                s_sb = cpool.tile([128, 128], f16, tag=f"s{dy}", name=f"s{dy}")
                nc.sync.dma_start(out=s_sb, in_=shift_dram[dy].ap())
                wS[dy] = cpool.tile([128, 128], f16, tag=f"ws{dy}",
                                    name=f"ws{dy}")
                nc.vector.tensor_scalar_mul(wS[dy], s_sb, w_sb[:, 0:1])
                M[dy] = cpool.tile([128, 128], f16, tag=f"m{dy}",
                                   name=f"m{dy}")
                nc.vector.tensor_tensor(M[dy], wS[dy], wi_sb, ADD)

            zt = cpool.tile([128, 256], f16, tag="zt", name="zt")
            nc.gpsimd.memset(zt, 0.0)

            def zmov(cw, ec):
                # arbitrary zero-valued moving operand of shape (cw, ec)
                return zt.rearrange("p (a b) -> p a b", b=ec)[:, 0:cw, :]

            xts = {}

            def mm(ps, po, tlo, thi, stat, src3, s_tok, c0, cw):
                """psum[:, (c: cw @po stride GT), (t: tlo..thi)] =
                   stat.T @ src3[:, s_tok.., c0:c0+cw] (moving dims (c, t))."""
                out = ps.rearrange("p (c t) -> p c t", t=GT)[
                    :, po // GT:po // GT + cw, tlo:thi]
                if src3 is None:
                    mov = zmov(cw, thi - tlo)
                else:
                    mov = src3[:, s_tok:s_tok + (thi - tlo),
                               c0:c0 + cw].transpose([0, 2, 1])
                nc.tensor.matmul(out, stat, mov, start=True, stop=True)

            def compute(k):
                xt = xts[k]
                xt3 = xt.rearrange("p (t c) -> p t c", c=C)
                prev3 = (xts[k - 1].rearrange("p (t c) -> p t c", c=C)
                         if k > 0 else None)
                next3 = (xts[k + 1].rearrange("p (t c) -> p t c", c=C)
                         if k < NCHUNK - 1 else None)

                ot = opool.tile([128, FD], f16, tag="ot", name="ot")
                ot3 = ot.rearrange("p (t c) -> p t c", c=C)

                wss = []
                # DVE @4x: prefill (1-w)*x on A + diagonal channel regions
                for (c0, cw) in ((0, 128), (256, 64), (DIAG_C0, DIAG_CW)):
                    nc.vector.tensor_scalar_mul(
                        ot3[:, :, c0:c0 + cw], xt3[:, :, c0:c0 + cw],
                        w1_sb[:, 0:1])

                for g in range(NG):
                    t0 = g * GT
                    psb1 = psb1pool.tile([128, B1_FD], f32, tag="b1", name="b1")
                    psb2 = psb2pool.tile([128, B2_FD], f32, tag="b2", name="b2")
                    ws = wspool.tile([128, GT * DIAG_CW], f16, tag="ws",
                                     name="ws")
                    ws3 = ws.rearrange("p (t c) -> p t c", c=DIAG_CW)
                    regions = {"b1": psb1, "b2": psb2}
                    # folded slabs: PSUM = w*shifted + (1-w)*x, final values
                    for (dy, dx, c0, cw) in BF_SLABS:
                        reg, po = _psum_region(c0)
                        mm(regions[reg], po, 0, GT, M[dy], xt3, t0, c0, cw)
                    # diagonal slabs: PSUM = w*shifted
                    for (dy, dx, c0, cw) in BD_SLABS:
                        reg, po = _psum_region(c0)
                        ps = regions[reg]
                        # token i (in group) sources chunk-token t0+i-dx
                        lo = max(0, dx - t0)               # from prev chunk
                        hi = min(GT, CHUNK + dx - t0)      # above: next chunk
                        if hi > lo:
                            mm(ps, po, lo, hi, wS[dy], xt3,
                               t0 + lo - dx, c0, cw)
                        if lo > 0:
                            mm(ps, po, 0, lo, wS[dy], prev3,
                               CHUNK + t0 - dx, c0, cw)
                        if hi < GT:
                            mm(ps, po, hi, GT, wS[dy], next3,
                               t0 + hi - dx - CHUNK, c0, cw)
                    # ACT: copy PSUM regions out (fp16 downcast)
                    p1v = psb1.rearrange("p (c t) -> p t c", t=GT)
                    p2v = psb2.rearrange("p (c t) -> p t c", t=GT)
                    # psb2-reading copies first: the next group's b2 matmuls
                    # wait on psb2's release, so the b1 copy (double-buffered
                    # region, nothing waiting) goes last
                    nc.scalar.copy(out=ws3[:, 0:GT, :],
                                   in_=p2v[:, :, 64:256])
                    nc.scalar.copy(out=ot3[:, t0:t0 + GT, 320:384],
                                   in_=p2v[:, :, 0:64])
                    nc.scalar.copy(out=ot3[:, t0:t0 + GT, 128:256],
                                   in_=p1v)
                    wss.append(ws3)

                # dy==0 slabs: in-place stt blend from dx-shifted SBUF views
                for (dy, dx, c0, cw) in A_SLABS:
                    lo = max(0, dx)
                    hi = CHUNK + min(0, dx)
                    dst = ot3[:, lo:hi, c0:c0 + cw]
                    src = xt3[:, lo - dx:hi - dx, c0:c0 + cw]
                    nc.vector.scalar_tensor_tensor(
                        out=dst, in0=src, scalar=w_sb[:, 0:1], in1=dst,
                        op0=MULT, op1=ADD)
                    if dx > 0 and prev3 is not None:
                        dst = ot3[:, 0:dx, c0:c0 + cw]
                        src = prev3[:, CHUNK - dx:CHUNK, c0:c0 + cw]
                        nc.vector.scalar_tensor_tensor(
                            out=dst, in0=src, scalar=w_sb[:, 0:1], in1=dst,
                            op0=MULT, op1=ADD)
                    elif dx < 0 and next3 is not None:
                        dst = ot3[:, CHUNK + dx:CHUNK, c0:c0 + cw]
                        src = next3[:, 0:-dx, c0:c0 + cw]
                        nc.vector.scalar_tensor_tensor(
                            out=dst, in0=src, scalar=w_sb[:, 0:1], in1=dst,
                            op0=MULT, op1=ADD)
                    # x-wrap edge (k==0 for dx>0, k==NCHUNK-1 for dx<0):
                    # shifted is 0 there; ot already holds (1-w)*x -> no op.

                # DVE @2x: diag out += ws (in place over (1-w)x prefill);
                # per group so the add fires as soon as that group's PSUM
                # copy lands, emitted after the stt blends to avoid DVE HOL
                for g in range(NG):
                    t0 = g * GT
                    nc.vector.tensor_tensor(
                        ot3[:, t0:t0 + GT, DIAG_C0:DIAG_C0 + DIAG_CW],
                        ot3[:, t0:t0 + GT, DIAG_C0:DIAG_C0 + DIAG_CW],
                        wss[g][:, 0:GT, :], ADD)

                # two half-chunk out-DMAs: the first half's writers finish
                # mid-chunk (group 0 copies/add + full-chunk DVE ops), so its
                # drain starts ~3-4us before the whole tile is done
                hf = FD // 2
                nc.gpsimd.dma_start(out=o_row[:, k * FD:k * FD + hf],
                                    in_=ot[:, 0:hf])
                nc.gpsimd.dma_start(out=o_row[:, k * FD + hf:(k + 1) * FD],
                                    in_=ot[:, hf:FD])

            # split input loads across both hardware DMA queues (sync +
            # scalar) so early chunks land ~2x sooner and compute starts fast
            for k in range(NCHUNK):
                xts[k] = xpool.tile([128, FD], f16, tag="xt", name="xt")
                q = nc.sync if k % 2 == 0 else nc.scalar
                q.dma_start(out=xts[k], in_=x_row[:, k * FD:(k + 1) * FD])
            for k in range(NCHUNK):
                compute(k)

    nc.compile()
    return nc


def _get_nc():
    if "nc" not in _CACHE:
        _CACHE["nc"] = _build_bass()
    return _CACHE["nc"]


def _run(x: np.ndarray, weight: np.ndarray, trace: bool = False, **kw):
    from concourse.bass_utils import run_bass_kernel_spmd

    nc = _get_nc()
    w_tile = np.full((128, 1), np.float32(weight), dtype=np.float32)
    in_maps = [
        {"x": np.ascontiguousarray(x[i]).astype(IO_NP_DT), "weight": w_tile}
        for i in range(NCORES)
    ]
    res = run_bass_kernel_spmd(
        nc, in_maps, core_ids=list(range(NCORES)), trace=trace, **kw)
    out = np.stack([r["out"].astype(np.float32) for r in res.results], axis=0)
    return out, res


def kernel(x: np.ndarray, weight: np.ndarray) -> np.ndarray:
    out, _ = _run(x, weight)
    return out
